# revision 1
# baseline (speedup 1.0000x reference)
"""F2NetHead Trainium2 kernel (8 NeuronCores, Bass/Tile).

Reference computation (per batch b):
    qog = x @ W_qog.T + b_qog ; Q,O,G = split(qog)
    cq  = silu(conv1d(Q, conv_w, pad=1) + conv_b)          # mixes channels
    l   = (cq @ w_a.T) / sqrt(d)
    attn= softmax(l, axis=seq)
    glob= sum_seq(Q * attn)                                 # [1, d]
    P   = O * glob
    L   = silu(G) * cumsum(P, axis=seq)
    R   = L @ W_out.T + b_out

Sharding: 8 cores = 4 batches x 2 sequence halves. Each core computes
2048 tokens of one batch. The host supplies the x-rows with a 1-token
halo on each side (zero rows at the sequence edges) so the conv needs no
neighbor exchange. The only cross-core communication is a pairwise
AllReduce of 3 small [d] vectors per batch:
    E  = sum_seq exp(l)            (softmax denominator)
    N  = sum_seq Q * exp(l)        (softmax numerator of glob)
    sx = sum of this half's x rows (first half only)
The cumsum offset of the second half is glob * (W_O @ sx_h0 + T*b_O),
i.e. the first half's P-column-sums, computed without materializing O.

On-chip layout is feature-major ([d partitions, tokens free]) so every
sequence-axis op (softmax sums, global sum, cumsum) is a free-dim op.
All matmuls run in float32r (full PE rate, ~1.6e-4 rel err on hw);
silu is computed as x*sigmoid(x) (ACT sigmoid + DVE multiply).
"""

import numpy as np

import concourse.bacc as bacc
import concourse.mybir as mybir
import concourse.tile as tile
from concourse.bass_utils import run_bass_kernel_spmd

F32 = mybir.dt.float32
F32R = mybir.dt.float32r
AF = mybir.ActivationFunctionType
OP = mybir.AluOpType

B, S, D, DM = 4, 4096, 1024, 1024
N_CORES = 8
T = S // 2            # tokens per core
TH = T + 2            # with halo
DT = D // 128         # d tiles (8)
KT = DM // 128        # contraction tiles (8)
ABLK = 410            # phase A token block (5 blocks over TH=2050)
BBLK = 512            # phase B token block (4 blocks over T)
CBLK = 256            # phase C token block (8 blocks over T)
SCALE = 1.0 / float(np.sqrt(D))


def _emit(tc, nc, prm, phases=5):
    reps = 1
    if phases >= 100:
        reps, phases = phases // 100, 5
    for _ in range(reps):
        _emit_once(tc, nc, prm, phases)


def _emit_once(tc, nc, prm, phases):
    x, wqt, wct, wat, wot = prm["x"], prm["wqt"], prm["wct"], prm["wat"], prm["wot"]
    bq, bo, bg, cb, bout = prm["bq"], prm["bo"], prm["bg"], prm["cb"], prm["bout"]
    hf0, hf1, r_out = prm["hf0"], prm["hf1"], prm["r"]

    with (
        tc.tile_pool(name="cols", bufs=1) as cols,
        tc.tile_pool(name="woo", bufs=1) as woo_pool,
        tc.tile_pool(name="dram", bufs=1, space="DRAM") as dram,
    ):
        # W_O^T loaded up-front (fits alongside every phase) so phase C's
        # O-matmuls and the offset matvec start right after the allreduce
        # instead of stalling on an 8 MiB weight load
        woo = woo_pool.tile([128, KT, DT, 128], F32R)
        for kc in range(KT):
            nc.sync.dma_start(
                woo[:, kc, :, :],
                wqt[kc * 128:(kc + 1) * 128, D:2 * D]
                .rearrange("p (a m) -> p a m", m=128).bitcast(F32R),
            )
        # per-partition bias / flag columns ([128, DT] with d = a*128 + p)
        bq_sb = cols.tile([128, DT], F32)
        bo_sb = cols.tile([128, DT], F32)
        bg_sb = cols.tile([128, DT], F32)
        cb_sb = cols.tile([128, DT], F32)
        bout_sb = cols.tile([128, DT], F32)
        hf0_sb = cols.tile([128, 1], F32)
        hf1_sb = cols.tile([128, 1], F32)
        for t_, d_ in ((bq_sb, bq), (bo_sb, bo), (bg_sb, bg), (cb_sb, cb),
                       (bout_sb, bout), (hf0_sb, hf0), (hf1_sb, hf1)):
            nc.sync.dma_start(t_[:], d_[:])

        # accumulators that survive across phases
        sx_cols = cols.tile([128, KT, 5], F32)      # per-A-block x sums
        e_cols = cols.tile([128, DT * 4], F32)      # per-(a,B-block) exp sums
        n_cols = cols.tile([128, DT * 4], F32)      # per-(a,B-block) Q*exp sums
        stage = cols.tile([128, 3 * DT], F32)       # allreduce staging
        red = cols.tile([128, 3 * DT], F32)         # allreduce result
        glob = cols.tile([128, DT], F32)
        offset = cols.tile([128, DT], F32)

        # ---------------- phase A: Q^T over TH halo'd tokens ----------------
        with tc.tile_pool(name="qt", bufs=1) as qt_pool:
            qt = qt_pool.tile([128, DT, TH], F32R)
            with (
                tc.tile_pool(name="wq", bufs=1) as wq_pool,
                tc.tile_pool(name="xa", bufs=2) as xa_pool,
                tc.tile_pool(name="psa", bufs=8, space="PSUM") as psa,
            ):
                wq = [wq_pool.tile([128, DT, 128], F32R, tag=f"wq{kc}",
                                   name=f"wq{kc}") for kc in range(KT)]
                for kc in range(KT):
                    nc.sync.dma_start(
                        wq[kc][:],
                        wqt[kc * 128:(kc + 1) * 128, 0:D]
                        .rearrange("p (a m) -> p a m", m=128).bitcast(F32R),
                    )
                for blk in range(5):
                    t0 = blk * ABLK
                    xt = [xa_pool.tile([128, ABLK], F32R, tag=f"xa{kc}",
                                       name=f"xa{kc}") for kc in range(KT)]
                    for kc in range(KT):
                        nc.sync.dma_start(
                            xt[kc][:],
                            x[kc * 128:(kc + 1) * 128, t0:t0 + ABLK].bitcast(F32R),
                        )
                    # x column-sums over main (non-halo) tokens for cumsum offset
                    lo = 1 - t0 if t0 < 1 else 0
                    hi = ABLK - max(0, t0 + ABLK - (TH - 1))
                    for kc in range(KT):
                        nc.vector.tensor_reduce(
                            sx_cols[:, kc, blk:blk + 1], xt[kc][:, lo:hi],
                            axis=mybir.AxisListType.X, op=OP.add,
                        )
                    for a in range(DT):
                        ps = psa.tile([128, ABLK], F32, tag="ps")
                        for kc in range(KT):
                            nc.tensor.matmul(
                                ps[:], wq[kc][:, a, :], xt[kc][:],
                                start=(kc == 0), stop=(kc == KT - 1),
                            )
                        nc.vector.tensor_scalar_add(
                            qt[:, a, t0:t0 + ABLK], ps[:], bq_sb[:, a:a + 1]
                        )

            if phases == 1:
                for a in range(DT):
                    nc.sync.dma_start(
                        r_out[a * 128:(a + 1) * 128, 0:T],
                        qt[:, a, 1:T + 1].bitcast(F32),
                    )
                return

            # ------------- phase B1: cq^T = silu(conv(Q)) -------------
            with tc.tile_pool(name="cq", bufs=1) as cq_pool:
                cq = cq_pool.tile([128, DT, T], F32R)
                with (
                    tc.tile_pool(name="wc", bufs=2) as wc_pool,
                    tc.tile_pool(name="psb", bufs=8, space="PSUM") as psb,
                ):
                    for a in range(DT):
                        wc = wc_pool.tile([128, 3, KT, 128], F32R, tag="wc")
                        for k3 in range(3):
                            nc.sync.dma_start(
                                wc[:, k3, :, :],
                                wct[k3, :, a * 128:(a + 1) * 128]
                                .rearrange("(kc p) m -> p kc m", p=128)
                                .bitcast(F32R),
                            )
                        for blk in range(T // BBLK):
                            t0 = blk * BBLK
                            ps = psb.tile([128, BBLK], F32, tag="ps")
                            first = True
                            for k3 in range(3):
                                for kc in range(KT):
                                    nc.tensor.matmul(
                                        ps[:], wc[:, k3, kc, :],
                                        qt[:, kc, t0 + k3:t0 + k3 + BBLK],
                                        start=first,
                                        stop=(k3 == 2 and kc == KT - 1),
                                    )
                                    first = False
                            sig = wc_pool.tile([128, BBLK], F32, tag="sig")
                            nc.scalar.activation(
                                sig[:], ps[:], AF.Sigmoid, bias=cb_sb[:, a:a + 1]
                            )
                            nc.vector.scalar_tensor_tensor(
                                cq[:, a, t0:t0 + BBLK], ps[:], cb_sb[:, a:a + 1],
                                sig[:], OP.add, OP.mult,
                            )

                if phases == 2:
                    for a in range(DT):
                        nc.sync.dma_start(
                            r_out[a * 128:(a + 1) * 128, 0:T],
                            cq[:, a, :].bitcast(F32),
                        )
                    return

                # ------- phase B2: E/N partial sums from exp(logits) -------
                with (
                    tc.tile_pool(name="wa", bufs=1) as wa_pool,
                    tc.tile_pool(name="ex", bufs=2) as ex_pool,
                    tc.tile_pool(name="psl", bufs=8, space="PSUM") as psl,
                ):
                    wa = [wa_pool.tile([128, DT, 128], F32R, tag=f"wa{kc}",
                                       name=f"wa{kc}") for kc in range(KT)]
                    for kc in range(KT):
                        nc.sync.dma_start(
                            wa[kc][:],
                            wat[kc * 128:(kc + 1) * 128, :]
                            .rearrange("p (a m) -> p a m", m=128).bitcast(F32R),
                        )
                    for blk in range(T // BBLK):
                        t0 = blk * BBLK
                        for a in range(DT):
                            ps = psl.tile([128, BBLK], F32, tag="ps")
                            for kc in range(KT):
                                nc.tensor.matmul(
                                    ps[:], wa[kc][:, a, :], cq[:, kc, t0:t0 + BBLK],
                                    start=(kc == 0), stop=(kc == KT - 1),
                                )
                            expl = ex_pool.tile([128, BBLK], F32, tag="expl")
                            idx = a * 4 + blk
                            nc.scalar.activation(
                                expl[:], ps[:], AF.Exp, scale=SCALE,
                                accum_out=e_cols[:, idx:idx + 1],
                            )
                            prod = ex_pool.tile([128, BBLK], F32, tag="prod")
                            nc.vector.scalar_tensor_tensor(
                                prod[:], expl[:], 0.0,
                                qt[:, a, t0 + 1:t0 + 1 + BBLK].bitcast(F32),
                                OP.add, OP.mult,
                                accum_out=n_cols[:, idx:idx + 1],
                            )

        if phases == 3:
            nc.sync.dma_start(r_out[0:128, 0:32].rearrange("p t -> p t"), e_cols[:])
            nc.sync.dma_start(r_out[128:256, 0:32], n_cols[:])
            return

        # ---------------- allreduce E, N, sx over the seq pair ----------------
        nc.vector.tensor_reduce(
            stage[:, 0:DT], e_cols[:].rearrange("p (a b) -> p a b", b=4),
            axis=mybir.AxisListType.X, op=OP.add,
        )
        nc.vector.tensor_reduce(
            stage[:, DT:2 * DT], n_cols[:].rearrange("p (a b) -> p a b", b=4),
            axis=mybir.AxisListType.X, op=OP.add,
        )
        # x sums (main tokens) masked to the first half: slot = sx * (1-h)
        nc.vector.tensor_reduce(
            stage[:, 2 * DT:3 * DT], sx_cols[:],
            axis=mybir.AxisListType.X, op=OP.add,
        )
        nc.vector.tensor_scalar_mul(
            stage[:, 2 * DT:3 * DT], stage[:, 2 * DT:3 * DT], hf0_sb[:, 0:1]
        )
        if phases == 99:
            # timing-model variant: skip the collective (TimelineSim
            # cannot model collectives); copy stage -> red locally
            nc.vector.tensor_copy(red[:], stage[:])
        else:
            cc_in = dram.tile([128, 3 * DT], F32)
            cc_out = dram.tile([128, 3 * DT], F32)
            nc.sync.dma_start(cc_in[:], stage[:])
            nc.gpsimd.collective_compute(
                "AllReduce", OP.add,
                replica_groups=[[0, 1], [2, 3], [4, 5], [6, 7]],
                ins=[cc_in.opt()], outs=[cc_out.opt()],
            )
            nc.sync.dma_start(red[:], cc_out[:])

        # glob = N / E
        recip = cols.tile([128, DT], F32)
        nc.vector.reciprocal(recip[:], red[:, 0:DT])
        nc.vector.tensor_mul(glob[:], red[:, DT:2 * DT], recip[:])

        # ---------------- phase C: O,G -> P -> cumsum -> L -> R ----------------
        with (
            tc.tile_pool(name="wog", bufs=1) as wog_pool,
            tc.tile_pool(name="wo2", bufs=1) as wo2_pool,
        ):
            wog = wog_pool.tile([128, KT, DT, 128], F32R)
            for kc in range(KT):
                nc.sync.dma_start(
                    wog[:, kc, :, :],
                    wqt[kc * 128:(kc + 1) * 128, 2 * D:3 * D]
                    .rearrange("p (a m) -> p a m", m=128).bitcast(F32R),
                )
            wo2 = wo2_pool.tile([128, KT, DT, 128], F32R)
            for kc in range(KT):
                nc.sync.dma_start(
                    wo2[:, kc, :, :],
                    wot[kc * 128:(kc + 1) * 128, :]
                    .rearrange("p (a m) -> p a m", m=128).bitcast(F32R),
                )

            # cumsum offset for the second half: glob * (W_O @ sx_h0 + T*b_O)
            # (plain fp32 matmul: fp32r rejects a size-1 moving operand)
            bo_t = cols.tile([128, DT], F32)
            nc.vector.tensor_scalar_mul(bo_t[:], bo_sb[:], float(T))
            offv = cols.tile([128, DT], F32)
            with tc.tile_pool(name="psm", bufs=2, space="PSUM") as psm:
                for a in range(DT):
                    ps = psm.tile([128, 1], F32, tag="ps")
                    for kc in range(KT):
                        nc.tensor.matmul(
                            ps[:], woo[:, kc, a, :].bitcast(F32),
                            red[:, 2 * DT + kc:2 * DT + kc + 1],
                            start=(kc == 0), stop=(kc == KT - 1),
                        )
                    nc.vector.tensor_scalar_add(
                        offv[:, a:a + 1], ps[:], bo_t[:, a:a + 1]
                    )
            nc.vector.tensor_mul(offset[:], offv[:], glob[:])
            nc.vector.tensor_scalar_mul(offset[:], offset[:], hf1_sb[:, 0:1])
            boglob = cols.tile([128, DT], F32)
            nc.vector.tensor_mul(boglob[:], bo_sb[:], glob[:])

            if phases == 4:
                nc.sync.dma_start(r_out[0:128, 0:DT], offset[:])
                nc.sync.dma_start(r_out[128:256, 0:DT], glob[:])
                return

            with (
                tc.tile_pool(name="xc", bufs=2) as xc_pool,
                tc.tile_pool(name="blkb", bufs=2) as blk_pool,
                tc.tile_pool(name="psc", bufs=8, space="PSUM") as psc,
            ):
                c_prev = None
                nblk = T // CBLK
                if 50 <= phases < 99:
                    nblk = phases - 50
                for blk in range(nblk):
                    t0 = blk * CBLK
                    xt = xc_pool.tile([128, KT, CBLK], F32R, tag="xc")
                    for kc in range(KT):
                        nc.sync.dma_start(
                            xt[:, kc, :],
                            x[kc * 128:(kc + 1) * 128, t0 + 1:t0 + 1 + CBLK]
                            .bitcast(F32R),
                        )
                    pt = blk_pool.tile([128, DT, CBLK], F32, tag="pt")
                    ct = blk_pool.tile([128, DT, CBLK], F32, tag="ct")
                    carry = xc_pool.tile([128, DT], F32, tag="carry")
                    gt = blk_pool.tile([128, DT, CBLK], F32, tag="gt")
                    lt = blk_pool.tile([128, DT, CBLK], F32R, tag="lt")
                    rt = blk_pool.tile([128, DT, CBLK], F32, tag="rt")
                    for a in range(DT):
                        ps = psc.tile([128, CBLK], F32, tag="ps")
                        for kc in range(KT):
                            nc.tensor.matmul(
                                ps[:], woo[:, kc, a, :], xt[:, kc, :],
                                start=(kc == 0), stop=(kc == KT - 1),
                            )
                        # P = (O + b_o) * glob = O*glob + (b_o*glob), on ACT
                        nc.scalar.activation(
                            pt[:, a, :], ps[:], AF.Identity,
                            bias=boglob[:, a:a + 1], scale=glob[:, a:a + 1],
                        )
                        init = (offset[:, a:a + 1] if c_prev is None
                                else c_prev[:, a:a + 1])
                        nc.vector.tensor_tensor_scan(
                            ct[:, a, :], pt[:, a, :], pt[:, a, :], init,
                            OP.add, OP.bypass,
                        )
                    # carry the last cumsum column via ACT so the next
                    # block's scan does not read a scan output directly
                    nc.scalar.copy(carry[:], ct[:, :, CBLK - 1:CBLK])
                    for a in range(DT):
                        ps = psc.tile([128, CBLK], F32, tag="ps")
                        for kc in range(KT):
                            nc.tensor.matmul(
                                ps[:], wog[:, kc, a, :], xt[:, kc, :],
                                start=(kc == 0), stop=(kc == KT - 1),
                            )
                        sig = xc_pool.tile([128, CBLK], F32, tag="sig")
                        nc.scalar.activation(
                            sig[:], ps[:], AF.Sigmoid, bias=bg_sb[:, a:a + 1]
                        )
                        nc.vector.scalar_tensor_tensor(
                            gt[:, a, :], ps[:], bg_sb[:, a:a + 1], sig[:],
                            OP.add, OP.mult,
                        )
                        nc.vector.tensor_mul(lt[:, a, :], gt[:, a, :], ct[:, a, :])
                    for a in range(DT):
                        ps = psc.tile([128, CBLK], F32, tag="ps")
                        for kc in range(KT):
                            nc.tensor.matmul(
                                ps[:], wo2[:, kc, a, :], lt[:, kc, :],
                                start=(kc == 0), stop=(kc == KT - 1),
                            )
                        nc.scalar.activation(
                            rt[:, a, :], ps[:], AF.Identity,
                            bias=bout_sb[:, a:a + 1],
                        )
                    for a in range(DT):
                        nc.sync.dma_start(
                            r_out[a * 128:(a + 1) * 128, t0:t0 + CBLK],
                            rt[:, a, :],
                        )
                    c_prev = carry


_CACHE = {}


def _build(phases=5):
    if phases in _CACHE:
        return _CACHE[phases]
    nc = bacc.Bacc(None, target_bir_lowering=False, num_devices=N_CORES)
    prm = {
        "x": nc.declare_dram_parameter("x", [DM, TH], F32, isOutput=False),
        "wqt": nc.declare_dram_parameter("wqt", [DM, 3 * D], F32, isOutput=False),
        "wct": nc.declare_dram_parameter("wct", [3, D, D], F32, isOutput=False),
        "wat": nc.declare_dram_parameter("wat", [D, D], F32, isOutput=False),
        "wot": nc.declare_dram_parameter("wot", [D, D], F32, isOutput=False),
        "bq": nc.declare_dram_parameter("bq", [128, DT], F32, isOutput=False),
        "bo": nc.declare_dram_parameter("bo", [128, DT], F32, isOutput=False),
        "bg": nc.declare_dram_parameter("bg", [128, DT], F32, isOutput=False),
        "cb": nc.declare_dram_parameter("cb", [128, DT], F32, isOutput=False),
        "bout": nc.declare_dram_parameter("bout", [128, DT], F32, isOutput=False),
        "hf0": nc.declare_dram_parameter("hf0", [128, 1], F32, isOutput=False),
        "hf1": nc.declare_dram_parameter("hf1", [128, 1], F32, isOutput=False),
        "r": nc.declare_dram_parameter("r", [DM, T], F32, isOutput=True),
    }
    with tile.TileContext(nc, num_cores=N_CORES) as tc:
        _emit(tc, nc, prm, phases)
    nc.compile()
    _CACHE[phases] = nc
    return nc


def make_in_maps(x, W_qog, b_qog, conv_w, conv_b, w_a, W_out, b_out):
    f = np.float32
    x = np.asarray(x, f)
    wqt = np.ascontiguousarray(np.asarray(W_qog, f).T)          # [dm, 3d]
    wct = np.ascontiguousarray(np.asarray(conv_w, f).transpose(2, 1, 0))
    wat = np.ascontiguousarray(np.asarray(w_a, f).T)
    wot = np.ascontiguousarray(np.asarray(W_out, f).T)

    def col(v):  # [d] -> [128, DT] with d = a*128 + p
        return np.ascontiguousarray(np.asarray(v, f).reshape(DT, 128).T)

    b_qog = np.asarray(b_qog, f)
    bq, bo, bg = col(b_qog[:D]), col(b_qog[D:2 * D]), col(b_qog[2 * D:])
    cb, bout = col(conv_b), col(b_out)

    in_maps = []
    for c in range(N_CORES):
        b, h = c // 2, c % 2
        t0 = h * T
        xs = np.zeros((TH, DM), f)
        xs[1:T + 1] = x[b, t0:t0 + T]
        if t0 > 0:
            xs[0] = x[b, t0 - 1]
        if t0 + T < S:
            xs[T + 1] = x[b, t0 + T]
        xs = np.ascontiguousarray(xs.T)            # [DM, TH] feature-major
        in_maps.append({
            "x": xs, "wqt": wqt, "wct": wct, "wat": wat, "wot": wot,
            "bq": bq, "bo": bo, "bg": bg, "cb": cb, "bout": bout,
            "hf0": np.full((128, 1), 1.0 - h, f),
            "hf1": np.full((128, 1), float(h), f),
        })
    return in_maps


def kernel(x, W_qog, b_qog, conv_w, conv_b, w_a, W_out, b_out):
    nc = _build(5)
    in_maps = make_in_maps(x, W_qog, b_qog, conv_w, conv_b, w_a, W_out, b_out)
    res = None
    for attempt in range(3):
        try:
            res = run_bass_kernel_spmd(nc, in_maps, list(range(N_CORES)))
            break
        except Exception:
            # the execution path through the device bridge is occasionally
            # flaky (worker hangup); reset the backend and retry
            if attempt == 2:
                raise
            import jax

            try:
                jax.clear_backends()
            except Exception:
                pass
            import time

            time.sleep(5)
    out = np.empty((B, S, DM), np.float32)
    for c in range(N_CORES):
        b, h = c // 2, c % 2
        out[b, h * T:(h + 1) * T, :] = res.results[c]["r"].T
    return out



# revision 24
# speedup vs baseline: 8.2867x; 8.2867x over previous
"""F2NetHead Trainium2 kernel (8 NeuronCores, Bass/Tile).

Reference computation (per batch b):
    qog = x @ W_qog.T + b_qog ; Q,O,G = split(qog)
    cq  = silu(conv1d(Q, conv_w, pad=1) + conv_b)          # mixes channels
    l   = (cq @ w_a.T) / sqrt(d)
    attn= softmax(l, axis=seq)
    glob= sum_seq(Q * attn)                                 # [1, d]
    P   = O * glob
    L   = silu(G) * cumsum(P, axis=seq)
    R   = L @ W_out.T + b_out

Sharding: 8 cores = 4 batches x 2 sequence halves. Each core computes
2048 tokens of one batch. The host supplies the x-rows with a 1-token
halo on each side (zero rows at the sequence edges) so the conv needs no
neighbor exchange. The only cross-core communication is a pairwise
AllReduce of 3 small [d] vectors per batch:
    E    = sum_seq exp(l)            (softmax denominator)
    N    = sum_seq Q * exp(l)        (softmax numerator of glob)
    offv = hf0 * (W_O @ sx + T*b_O)  (first half's P-column-sums / glob)
The cumsum offset of the second half is glob * offv, computed BEFORE the
collective from the local x column-sums (masked to the first half) so the
tensor engine's program order never stalls on the allreduce: after the
B2 matmuls it proceeds straight into phase C's O/G matmuls, which only
need weights prefetched long before.

On-chip layout is feature-major ([d partitions, tokens free]) so every
sequence-axis op (softmax sums, global sum, cumsum) is a free-dim op.
All matmul operands are bf16 (same PE rate as fp32r but half the DMA
and SBUF footprint, which is what lets every weight prefetch early and
x stay resident); all accumulations (PSUM, softmax sums, cumsum) are
fp32. Phase C is software-pipelined one block deep (PE order per block:
O_i, R_{i-1}, G_i) so the output matmul never waits on the silu/cumsum
chain of its own block.
"""

import numpy as np
import ml_dtypes

import concourse.bacc as bacc
import concourse.mybir as mybir
import concourse.tile as tile
from concourse.bass_utils import run_bass_kernel_spmd

F32 = mybir.dt.float32
BF16 = mybir.dt.bfloat16
AF = mybir.ActivationFunctionType
OP = mybir.AluOpType

B, S, D, DM = 4, 4096, 1024, 1024
N_CORES = 8
T = S // 2            # tokens per core
TH = T + 2            # with halo
DT = D // 128         # d tiles (8)
KT = DM // 128        # contraction tiles (8)
ABLK = 410            # phase A token block (5 blocks over TH=2050)
BBLK = 512            # phase B token block (4 blocks over T)
CBLK = 512            # phase C token block (4 blocks over T)
SCALE = 1.0 / float(np.sqrt(D))


def _emit(tc, nc, prm, phases=5):
    reps = 1
    if phases >= 100:
        reps, phases = phases // 100, 5
    for _ in range(reps):
        _emit_once(tc, nc, prm, phases)


def _emit_once(tc, nc, prm, phases):
    x, wqt, wct, wat, wot = prm["x"], prm["wqt"], prm["wct"], prm["wat"], prm["wot"]
    bq, hf0, r_out = prm["bcol"], prm["hf"], prm["r"]

    with (
        tc.tile_pool(name="cols", bufs=1) as cols,
        tc.tile_pool(name="xres", bufs=1) as x_pool,
        tc.tile_pool(name="woo", bufs=1) as woo_pool,
        tc.tile_pool(name="wog", bufs=1) as wog_pool,
        tc.tile_pool(name="wo2", bufs=1) as wo2_pool,
        tc.tile_pool(name="psu", bufs=8, space="PSUM") as psu,
        tc.tile_pool(name="dram", bufs=1, space="DRAM") as dram,
    ):
        # x stays resident for the whole kernel: phase A consumes it by
        # blocks, phase C's O/G matmuls reread it with no second DMA
        xa = x_pool.tile([128, KT, TH], BF16)
        woo = woo_pool.tile([128, KT, DT, 128], BF16)
        wog = wog_pool.tile([128, KT, DT, 128], BF16)
        wo2 = wo2_pool.tile([128, KT, DT, 128], BF16)

        # per-partition bias / flag columns ([128, DT] with d = a*128 + p),
        # packed [bq|bo|bg|cb|bout] so one DMA loads them all
        ball = cols.tile([128, 5 * DT], F32)
        bq_sb = ball[:, 0:DT]
        bo_sb = ball[:, DT:2 * DT]
        bg_sb = ball[:, 2 * DT:3 * DT]
        cb_sb = ball[:, 3 * DT:4 * DT]
        bout_sb = ball[:, 4 * DT:5 * DT]
        hf = cols.tile([128, 2], F32)
        hf0_sb = hf[:, 0:1]
        hf1_sb = hf[:, 1:2]

        # accumulators that survive across phases
        sx_cols = cols.tile([128, KT], F32)         # x column sums (main toks)
        sxb = cols.tile([128, KT], BF16)            # ... as matvec operand
        e_cols = cols.tile([128, DT * 4], F32)      # per-(a,B-block) exp sums
        n_cols = cols.tile([128, DT * 4], F32)      # per-(a,B-block) Q*exp sums
        stage = cols.tile([128, 3 * DT], F32)       # allreduce staging
        red = cols.tile([128, 3 * DT], F32)         # allreduce result
        glob = cols.tile([128, DT], F32)
        offset = cols.tile([128, DT], F32)
        boglob = cols.tile([128, DT], F32)

        # ---------------- phase A: Q^T over TH halo'd tokens ----------------
        # DMA queue order is emission order, so criticals go first: x block 0
        # and wq feed the first matmuls; everything phase C needs trickles in
        # behind the phase A stream.
        with tc.tile_pool(name="qt", bufs=1) as qt_pool:
            qt = qt_pool.tile([128, DT, TH], BF16)
            with tc.tile_pool(name="wq", bufs=1) as wq_pool:
                xr = x.rearrange("(kc p) t -> p kc t", p=128)
                wqr = wqt[:, 0:D].rearrange("(kc p) m -> p kc m", p=128)
                wq = wq_pool.tile([128, KT, DT * 128], BF16)
                # interleave the first x block with wq so the a=0 matmul
                # chain can start as soon as its first operands land
                for kc in range(0, KT, 2):
                    nc.sync.dma_start(
                        xa[:, kc:kc + 2, 0:ABLK], xr[:, kc:kc + 2, 0:ABLK]
                    )
                    nc.sync.dma_start(wq[:, kc:kc + 2, :], wqr[:, kc:kc + 2, :])
                    if kc == 2:
                        # biases aren't needed until the first qt write
                        nc.sync.dma_start(ball[:], bq[:])
                        nc.sync.dma_start(hf[:], hf0[:])
                for blk in range(5):
                    t0 = blk * ABLK
                    if blk > 0:
                        nc.sync.dma_start(
                            xa[:, :, t0:t0 + ABLK], xr[:, :, t0:t0 + ABLK]
                        )
                    for a in range(DT):
                        ps = psu.tile([128, ABLK], F32, tag="ps")
                        for kc in range(KT):
                            nc.tensor.matmul(
                                ps[:], wq[:, kc, a * 128:(a + 1) * 128],
                                xa[:, kc, t0:t0 + ABLK],
                                start=(kc == 0), stop=(kc == KT - 1),
                            )
                        nc.vector.tensor_scalar_add(
                            qt[:, a, t0:t0 + ABLK], ps[:], bq_sb[:, a:a + 1]
                        )
                    if blk == 0:
                        # phase C's O-projection weights ride behind block 0
                        nc.sync.dma_start(
                            woo[:].rearrange("p kc a m -> p kc (a m)"),
                            wqt[:, D:2 * D].rearrange("(kc p) m -> p kc m", p=128),
                        )
                # x column sums over main tokens, for the cumsum offset
                for kc in range(KT):
                    nc.vector.tensor_reduce(
                        sx_cols[:, kc:kc + 1], xa[:, kc, 1:T + 1],
                        axis=mybir.AxisListType.X, op=OP.add,
                    )
                nc.vector.tensor_copy(sxb[:], sx_cols[:])

            # ------------- phase B1: cq^T = silu(conv(Q)) -------------
            with (
                tc.tile_pool(name="cq", bufs=1) as cq_pool,
                tc.tile_pool(name="wa", bufs=1) as wa_pool,
            ):
                cq = cq_pool.tile([128, DT, T], BF16)
                wa = wa_pool.tile([128, KT, DT * 128], BF16)
                with (
                    tc.tile_pool(name="wc", bufs=2) as wc_pool,
                    tc.tile_pool(name="ex", bufs=2) as ex_pool,
                ):
                    for a in range(DT):
                        wc = wc_pool.tile([128, KT, 3 * 128], BF16, tag="wc")
                        nc.sync.dma_start(
                            wc[:],
                            wct[a].rearrange("(kc p) km -> p kc km", p=128),
                        )
                        if a == 0:
                            nc.sync.dma_start(
                                wog[:].rearrange("p kc a m -> p kc (a m)"),
                                wqt[:, 2 * D:3 * D]
                                .rearrange("(kc p) m -> p kc m", p=128),
                            )
                        if a == 2:
                            nc.sync.dma_start(
                                wa[:],
                                wat[:].rearrange("(kc p) m -> p kc m", p=128),
                            )
                        if a == 4:
                            nc.sync.dma_start(
                                wo2[:].rearrange("p kc a m -> p kc (a m)"),
                                wot[:].rearrange("(kc p) m -> p kc m", p=128),
                            )
                        for blk in range(T // BBLK):
                            t0 = blk * BBLK
                            ps = psu.tile([128, BBLK], F32, tag="ps")
                            first = True
                            for k3 in range(3):
                                for kc in range(KT):
                                    nc.tensor.matmul(
                                        ps[:],
                                        wc[:, kc, k3 * 128:(k3 + 1) * 128],
                                        qt[:, kc, t0 + k3:t0 + k3 + BBLK],
                                        start=first,
                                        stop=(k3 == 2 and kc == KT - 1),
                                    )
                                    first = False
                            sig = wc_pool.tile([128, BBLK], F32, tag="sig")
                            nc.scalar.activation(
                                sig[:], ps[:], AF.Sigmoid, bias=cb_sb[:, a:a + 1]
                            )
                            nc.vector.scalar_tensor_tensor(
                                cq[:, a, t0:t0 + BBLK], ps[:], cb_sb[:, a:a + 1],
                                sig[:], OP.add, OP.mult,
                            )

                    # ------- phase B2: E/N partial sums from exp(logits) ----
                    # (same PSUM pool as B1 so the bank rotation pipelines
                    # straight across the phase boundary)
                    for blk in range(T // BBLK):
                        t0 = blk * BBLK
                        for a in range(DT):
                            ps = psu.tile([128, BBLK], F32, tag="ps")
                            for kc in range(KT):
                                nc.tensor.matmul(
                                    ps[:], wa[:, kc, a * 128:(a + 1) * 128],
                                    cq[:, kc, t0:t0 + BBLK],
                                    start=(kc == 0), stop=(kc == KT - 1),
                                )
                            expl = ex_pool.tile([128, BBLK], F32, tag="expl")
                            idx = a * 4 + blk
                            nc.scalar.activation(
                                expl[:], ps[:], AF.Exp, scale=SCALE,
                                accum_out=e_cols[:, idx:idx + 1],
                            )
                            prod = ex_pool.tile([128, BBLK], BF16, tag="prod")
                            nc.vector.scalar_tensor_tensor(
                                prod[:], expl[:], 0.0,
                                qt[:, a, t0 + 1:t0 + 1 + BBLK],
                                OP.add, OP.mult,
                                accum_out=n_cols[:, idx:idx + 1],
                            )

        # -------- offv = hf0 * (W_O @ sx + T*b_O), before the collective ----
        # (uses only local sx, which is exact on first-half cores and masked
        # to zero on second-half ones)
        bo_t = cols.tile([128, DT], F32)
        nc.vector.tensor_scalar_mul(bo_t[:], bo_sb[:], float(T))
        if True:
            for a in range(DT):
                ps = psu.tile([128, 1], F32, tag="ps")
                for kc in range(KT):
                    nc.tensor.matmul(
                        ps[:], woo[:, kc, a, :], sxb[:, kc:kc + 1],
                        start=(kc == 0), stop=(kc == KT - 1),
                    )
                nc.vector.tensor_scalar_add(
                    stage[:, 2 * DT + a:2 * DT + a + 1], ps[:], bo_t[:, a:a + 1]
                )
        nc.vector.tensor_scalar_mul(
            stage[:, 2 * DT:3 * DT], stage[:, 2 * DT:3 * DT], hf0_sb[:, 0:1]
        )

        # ---------------- allreduce E, N, offv over the seq pair ----------------
        nc.vector.tensor_reduce(
            stage[:, 0:DT], e_cols[:].rearrange("p (a b) -> p a b", b=4),
            axis=mybir.AxisListType.X, op=OP.add,
        )
        nc.vector.tensor_reduce(
            stage[:, DT:2 * DT], n_cols[:].rearrange("p (a b) -> p a b", b=4),
            axis=mybir.AxisListType.X, op=OP.add,
        )
        if phases == 99:
            # timing-model variant: skip the collective (TimelineSim
            # cannot model collectives); copy stage -> red locally
            nc.vector.tensor_copy(red[:], stage[:])
        else:
            cc_in = dram.tile([128, 3 * DT], F32)
            cc_out = dram.tile([128, 3 * DT], F32)
            nc.sync.dma_start(cc_in[:], stage[:])
            nc.gpsimd.collective_compute(
                "AllReduce", OP.add,
                replica_groups=[[0, 1], [2, 3], [4, 5], [6, 7]],
                ins=[cc_in.opt()], outs=[cc_out.opt()],
            )
            nc.sync.dma_start(red[:], cc_out[:])

        # glob = N / E ; offset = glob * offv * hf1 ; boglob = b_o * glob
        recip = cols.tile([128, DT], F32)
        nc.vector.reciprocal(recip[:], red[:, 0:DT])
        nc.vector.tensor_mul(glob[:], red[:, DT:2 * DT], recip[:])
        nc.vector.tensor_mul(offset[:], red[:, 2 * DT:3 * DT], glob[:])
        nc.vector.tensor_scalar_mul(offset[:], offset[:], hf1_sb[:, 0:1])
        nc.vector.tensor_mul(boglob[:], bo_sb[:], glob[:])

        # ---------------- phase C: O,G -> P -> cumsum -> L -> R ----------------
        # software-pipelined: PE order per iteration is O_i, R_{i-1}, G_i so
        # the W_out matmul of block i runs while block i+1's silu/cumsum
        # chain completes on DVE/ACT
        with tc.tile_pool(name="blkb", bufs=2) as blk_pool:
            nblk = T // CBLK
            c_prev = None
            hist = []          # (lt, rt) of the previous block
            for blk in range(nblk):
                t0 = blk * CBLK
                pt = blk_pool.tile([128, DT, CBLK], F32, tag="pt")
                ct = blk_pool.tile([128, DT, CBLK], F32, tag="ct")
                carry = blk_pool.tile([128, DT], F32, tag="carry")
                gt = blk_pool.tile([128, DT, CBLK], BF16, tag="gt")
                lt = blk_pool.tile([128, DT, CBLK], BF16, tag="lt")
                # O-projection + P + cumsum for this block
                for a in range(DT):
                    ps = psu.tile([128, CBLK], F32, tag="ps")
                    for kc in range(KT):
                        nc.tensor.matmul(
                            ps[:], woo[:, kc, a, :], xa[:, kc, t0 + 1:t0 + 1 + CBLK],
                            start=(kc == 0), stop=(kc == KT - 1),
                        )
                    # P = (O + b_o) * glob = O*glob + (b_o*glob)
                    nc.vector.tensor_scalar(
                        pt[:, a, :], ps[:], glob[:, a:a + 1], boglob[:, a:a + 1],
                        OP.mult, OP.add,
                    )
                    init = (offset[:, a:a + 1] if c_prev is None
                            else c_prev[:, a:a + 1])
                    nc.vector.tensor_tensor_scan(
                        ct[:, a, :], pt[:, a, :], pt[:, a, :], init,
                        OP.add, OP.bypass,
                    )
                # carry the last cumsum column via ACT so the next
                # block's scan does not read a scan output directly
                nc.scalar.copy(carry[:], ct[:, :, CBLK - 1:CBLK])
                # output matmul of the PREVIOUS block
                if hist:
                    plt, prt, pt0 = hist.pop()
                    for a in range(DT):
                        ps = psu.tile([128, CBLK], F32, tag="ps")
                        for kc in range(KT):
                            nc.tensor.matmul(
                                ps[:], wo2[:, kc, a, :], plt[:, kc, :],
                                start=(kc == 0), stop=(kc == KT - 1),
                            )
                        nc.scalar.activation(
                            prt[:, a, :], ps[:], AF.Identity,
                            bias=bout_sb[:, a:a + 1],
                        )
                        nc.sync.dma_start(
                            r_out[a * 128:(a + 1) * 128, pt0:pt0 + CBLK],
                            prt[:, a, :],
                        )
                # G-projection + silu + L for this block
                for a in range(DT):
                    ps = psu.tile([128, CBLK], F32, tag="ps")
                    for kc in range(KT):
                        nc.tensor.matmul(
                            ps[:], wog[:, kc, a, :], xa[:, kc, t0 + 1:t0 + 1 + CBLK],
                            start=(kc == 0), stop=(kc == KT - 1),
                        )
                    sig = blk_pool.tile([128, CBLK], F32, tag="sig")
                    nc.scalar.activation(
                        sig[:], ps[:], AF.Sigmoid, bias=bg_sb[:, a:a + 1]
                    )
                    nc.vector.scalar_tensor_tensor(
                        gt[:, a, :], ps[:], bg_sb[:, a:a + 1], sig[:],
                        OP.add, OP.mult,
                    )
                    nc.vector.tensor_mul(lt[:, a, :], gt[:, a, :], ct[:, a, :])
                rt = blk_pool.tile([128, DT, CBLK], BF16, tag="rt")
                hist.append((lt, rt, t0))
                c_prev = carry
            # drain: output matmul of the final block; the bias add
            # alternates ACT/DVE so neither engine queues up a tail
            plt, prt, pt0 = hist.pop()
            for a in range(DT):
                ps = psu.tile([128, CBLK], F32, tag="ps")
                for kc in range(KT):
                    nc.tensor.matmul(
                        ps[:], wo2[:, kc, a, :], plt[:, kc, :],
                        start=(kc == 0), stop=(kc == KT - 1),
                    )
                if a % 2 == 0:
                    nc.scalar.activation(
                        prt[:, a, :], ps[:], AF.Identity,
                        bias=bout_sb[:, a:a + 1],
                    )
                else:
                    nc.vector.tensor_scalar_add(
                        prt[:, a, :], ps[:], bout_sb[:, a:a + 1]
                    )
                nc.sync.dma_start(
                    r_out[a * 128:(a + 1) * 128, pt0:pt0 + CBLK],
                    prt[:, a, :],
                )


_CACHE = {}


def _build(phases=5):
    if phases in _CACHE:
        return _CACHE[phases]
    nc = bacc.Bacc(None, target_bir_lowering=False, num_devices=N_CORES)
    prm = {
        "x": nc.declare_dram_parameter("x", [DM, TH], BF16, isOutput=False),
        "wqt": nc.declare_dram_parameter("wqt", [DM, 3 * D], BF16, isOutput=False),
        "wct": nc.declare_dram_parameter("wct", [DT, D, 3 * 128], BF16, isOutput=False),
        "wat": nc.declare_dram_parameter("wat", [D, D], BF16, isOutput=False),
        "wot": nc.declare_dram_parameter("wot", [D, D], BF16, isOutput=False),
        "bcol": nc.declare_dram_parameter("bcol", [128, 5 * DT], F32, isOutput=False),
        "hf": nc.declare_dram_parameter("hf", [128, 2], F32, isOutput=False),
        "r": nc.declare_dram_parameter("r", [DM, T], BF16, isOutput=True),
    }
    with tile.TileContext(nc, num_cores=N_CORES) as tc:
        _emit(tc, nc, prm, phases)
    nc.compile()
    _CACHE[phases] = nc
    return nc


def make_in_maps(x, W_qog, b_qog, conv_w, conv_b, w_a, W_out, b_out):
    f = np.float32
    bf = ml_dtypes.bfloat16
    x = np.asarray(x, f)
    wqt = np.ascontiguousarray(np.asarray(W_qog, f).T.astype(bf))   # [dm, 3d]
    # conv_w [o, i, k] -> wct [a, i, k*128+m] with o = a*128 + m, so one DMA
    # per output tile loads the whole 3-tap stationary slab
    wct = np.ascontiguousarray(
        np.asarray(conv_w, f)
        .reshape(DT, 128, D, 3)
        .transpose(0, 2, 3, 1)
        .reshape(DT, D, 3 * 128)
        .astype(bf)
    )
    wat = np.ascontiguousarray(np.asarray(w_a, f).T.astype(bf))
    wot = np.ascontiguousarray(np.asarray(W_out, f).T.astype(bf))

    def col(v):  # [d] -> [128, DT] with d = a*128 + p
        return np.asarray(v, f).reshape(DT, 128).T

    b_qog = np.asarray(b_qog, f)
    bcol = np.ascontiguousarray(np.concatenate(
        [col(b_qog[:D]), col(b_qog[D:2 * D]), col(b_qog[2 * D:]),
         col(conv_b), col(b_out)], axis=1))

    in_maps = []
    for c in range(N_CORES):
        b, h = c // 2, c % 2
        t0 = h * T
        xs = np.zeros((TH, DM), f)
        xs[1:T + 1] = x[b, t0:t0 + T]
        if t0 > 0:
            xs[0] = x[b, t0 - 1]
        if t0 + T < S:
            xs[T + 1] = x[b, t0 + T]
        xs = np.ascontiguousarray(xs.T.astype(bf))   # [DM, TH] feature-major
        hfv = np.zeros((128, 2), f)
        hfv[:, 0] = 1.0 - h
        hfv[:, 1] = float(h)
        in_maps.append({
            "x": xs, "wqt": wqt, "wct": wct, "wat": wat, "wot": wot,
            "bcol": bcol, "hf": hfv,
        })
    return in_maps


def kernel(x, W_qog, b_qog, conv_w, conv_b, w_a, W_out, b_out):
    nc = _build(5)
    in_maps = make_in_maps(x, W_qog, b_qog, conv_w, conv_b, w_a, W_out, b_out)
    res = None
    for attempt in range(3):
        try:
            res = run_bass_kernel_spmd(nc, in_maps, list(range(N_CORES)))
            break
        except Exception:
            # the execution path through the device bridge is occasionally
            # flaky (worker hangup); reset the backend and retry
            if attempt == 2:
                raise
            import jax

            try:
                jax.clear_backends()
            except Exception:
                pass
            import time

            time.sleep(5)
    out = np.empty((B, S, DM), np.float32)
    for c in range(N_CORES):
        b, h = c // 2, c % 2
        out[b, h * T:(h + 1) * T, :] = np.asarray(res.results[c]["r"], np.float32).T
    return out


# revision 26
# speedup vs baseline: 11.2625x; 1.3591x over previous
"""F2NetHead Trainium2 kernel (8 NeuronCores, Bass/Tile).

Reference computation (per batch b):
    qog = x @ W_qog.T + b_qog ; Q,O,G = split(qog)
    cq  = silu(conv1d(Q, conv_w, pad=1) + conv_b)          # mixes channels
    l   = (cq @ w_a.T) / sqrt(d)
    attn= softmax(l, axis=seq)
    glob= sum_seq(Q * attn)                                 # [1, d]
    P   = O * glob
    L   = silu(G) * cumsum(P, axis=seq)
    R   = L @ W_out.T + b_out

Sharding: 8 cores = 4 batches x 2 sequence halves. Each core computes
2048 tokens of one batch. The host supplies the x-rows with a 1-token
halo on each side (zero rows at the sequence edges) so the conv needs no
neighbor exchange. The only cross-core communication is a pairwise
AllReduce of 3 small [d] vectors per batch:
    E    = sum_seq exp(l)            (softmax denominator)
    N    = sum_seq Q * exp(l)        (softmax numerator of glob)
    offv = hf0 * (W_O @ sx + T*b_O)  (first half's P-column-sums / glob)
The cumsum offset of the second half is glob * offv, computed BEFORE the
collective from the local x column-sums (masked to the first half) so the
tensor engine's program order never stalls on the allreduce: after the
B2 matmuls it proceeds straight into phase C's O/G matmuls, which only
need weights prefetched long before.

On-chip layout is feature-major ([d partitions, tokens free]) so every
sequence-axis op (softmax sums, global sum, cumsum) is a free-dim op.
All matmul operands are bf16 (same PE rate as fp32r but half the DMA
and SBUF footprint, which is what lets every weight prefetch early and
x stay resident); all accumulations (PSUM, softmax sums, cumsum) are
fp32. Phase C is software-pipelined one block deep (PE order per block:
O_i, R_{i-1}, G_i) so the output matmul never waits on the silu/cumsum
chain of its own block.
"""

import numpy as np
import ml_dtypes

import concourse.bacc as bacc
import concourse.mybir as mybir
import concourse.tile as tile
from concourse.bass_utils import run_bass_kernel_spmd

F32 = mybir.dt.float32
BF16 = mybir.dt.bfloat16
AF = mybir.ActivationFunctionType
OP = mybir.AluOpType

B, S, D, DM = 4, 4096, 1024, 1024
N_CORES = 8
T = S // 2            # tokens per core
TH = T + 2            # with halo
DT = D // 128         # d tiles (8)
KT = DM // 128        # contraction tiles (8)
ABLK = 410            # phase A token block (5 blocks over TH=2050)
BBLK = 512            # phase B token block (4 blocks over T)
CBLK = 512            # phase C token block (4 blocks over T)
SCALE = 1.0 / float(np.sqrt(D))

# every input rides in one flat bf16 tensor (fewer per-launch dispatch args);
# region offsets in elements
OFF_X = 0                          # x        [DM, TH]
OFF_BH = OFF_X + DM * TH           # bcol|hf  [128, 5*DT + 2]
OFF_WQ = OFF_BH + 128 * (5 * DT + 2)   # W_qog^T  [DM, 3*D]
OFF_WC = OFF_WQ + DM * 3 * D       # conv     [DT, D, 3*128]
OFF_WA = OFF_WC + DT * D * 3 * 128  # w_a^T   [D, D]
OFF_WO = OFF_WA + D * D            # W_out^T  [D, D]
NWALL = OFF_WO + D * D


def _emit(tc, nc, prm, phases=5):
    reps = 1
    if phases >= 100:
        reps, phases = phases // 100, 5
    for _ in range(reps):
        _emit_once(tc, nc, prm, phases)


def _emit_once(tc, nc, prm, phases):
    wall, r_out = prm["wall"], prm["r"]
    xr = wall[OFF_X:OFF_BH].rearrange("(kc p t) -> p kc t", p=128, t=TH)
    bh = wall[OFF_BH:OFF_WQ].rearrange("(p c) -> p c", c=5 * DT + 2)
    wqt = wall[OFF_WQ:OFF_WC].rearrange("(i m) -> i m", m=3 * D)
    wct = wall[OFF_WC:OFF_WA].rearrange("(a i km) -> a i km", i=D, km=3 * 128)
    wat = wall[OFF_WA:OFF_WO].rearrange("(i m) -> i m", m=D)
    wot = wall[OFF_WO:NWALL].rearrange("(i m) -> i m", m=D)

    with (
        tc.tile_pool(name="cols", bufs=1) as cols,
        tc.tile_pool(name="xres", bufs=1) as x_pool,
        tc.tile_pool(name="woo", bufs=1) as woo_pool,
        tc.tile_pool(name="wog", bufs=1) as wog_pool,
        tc.tile_pool(name="wo2", bufs=1) as wo2_pool,
        tc.tile_pool(name="psu", bufs=8, space="PSUM") as psu,
        tc.tile_pool(name="dram", bufs=1, space="DRAM") as dram,
    ):
        # x stays resident for the whole kernel: phase A consumes it by
        # blocks, phase C's O/G matmuls reread it with no second DMA
        xa = x_pool.tile([128, KT, TH], BF16)
        woo = woo_pool.tile([128, KT, DT, 128], BF16)
        wog = wog_pool.tile([128, KT, DT, 128], BF16)
        wo2 = wo2_pool.tile([128, KT, DT, 128], BF16)

        # per-partition bias / flag columns ([128, DT] with d = a*128 + p),
        # packed [bq|bo|bg|cb|bout|hf0|hf1]: one DMA + one widening copy
        ball_bf = cols.tile([128, 5 * DT + 2], BF16)
        ball = cols.tile([128, 5 * DT + 2], F32)
        bq_sb = ball[:, 0:DT]
        bo_sb = ball[:, DT:2 * DT]
        bg_sb = ball[:, 2 * DT:3 * DT]
        cb_sb = ball[:, 3 * DT:4 * DT]
        bout_sb = ball[:, 4 * DT:5 * DT]
        hf0_sb = ball[:, 5 * DT:5 * DT + 1]
        hf1_sb = ball[:, 5 * DT + 1:5 * DT + 2]

        # accumulators that survive across phases
        sx_cols = cols.tile([128, KT], F32)         # x column sums (main toks)
        sxb = cols.tile([128, KT], BF16)            # ... as matvec operand
        e_cols = cols.tile([128, DT * 4], F32)      # per-(a,B-block) exp sums
        n_cols = cols.tile([128, DT * 4], F32)      # per-(a,B-block) Q*exp sums
        stage = cols.tile([128, 3 * DT], F32)       # allreduce staging
        red = cols.tile([128, 3 * DT], F32)         # allreduce result
        glob = cols.tile([128, DT], F32)
        offset = cols.tile([128, DT], F32)
        boglob = cols.tile([128, DT], F32)

        # ---------------- phase A: Q^T over TH halo'd tokens ----------------
        # DMA queue order is emission order, so criticals go first: x block 0
        # and wq feed the first matmuls; everything phase C needs trickles in
        # behind the phase A stream.
        with tc.tile_pool(name="qt", bufs=1) as qt_pool:
            qt = qt_pool.tile([128, DT, TH], BF16)
            with tc.tile_pool(name="wq", bufs=1) as wq_pool:
                wqr = wqt[:, 0:D].rearrange("(kc p) m -> p kc m", p=128)
                wq = wq_pool.tile([128, KT, DT * 128], BF16)
                # interleave the first x block with wq so the a=0 matmul
                # chain can start as soon as its first operands land
                for kc in range(0, KT, 2):
                    nc.sync.dma_start(
                        xa[:, kc:kc + 2, 0:ABLK], xr[:, kc:kc + 2, 0:ABLK]
                    )
                    nc.sync.dma_start(wq[:, kc:kc + 2, :], wqr[:, kc:kc + 2, :])
                    if kc == 2:
                        # biases aren't needed until the first qt write
                        nc.sync.dma_start(ball_bf[:], bh[:])
                        nc.vector.tensor_copy(ball[:], ball_bf[:])
                for blk in range(5):
                    t0 = blk * ABLK
                    if blk > 0:
                        nc.sync.dma_start(
                            xa[:, :, t0:t0 + ABLK], xr[:, :, t0:t0 + ABLK]
                        )
                    for a in range(DT):
                        ps = psu.tile([128, ABLK], F32, tag="ps")
                        for kc in range(KT):
                            nc.tensor.matmul(
                                ps[:], wq[:, kc, a * 128:(a + 1) * 128],
                                xa[:, kc, t0:t0 + ABLK],
                                start=(kc == 0), stop=(kc == KT - 1),
                            )
                        nc.vector.tensor_scalar_add(
                            qt[:, a, t0:t0 + ABLK], ps[:], bq_sb[:, a:a + 1]
                        )
                    if blk == 0:
                        # phase C's O-projection weights ride behind block 0
                        nc.sync.dma_start(
                            woo[:].rearrange("p kc a m -> p kc (a m)"),
                            wqt[:, D:2 * D].rearrange("(kc p) m -> p kc m", p=128),
                        )
                # x column sums over main tokens, for the cumsum offset
                for kc in range(KT):
                    nc.vector.tensor_reduce(
                        sx_cols[:, kc:kc + 1], xa[:, kc, 1:T + 1],
                        axis=mybir.AxisListType.X, op=OP.add,
                    )
                nc.vector.tensor_copy(sxb[:], sx_cols[:])

            # ------------- phase B1: cq^T = silu(conv(Q)) -------------
            with (
                tc.tile_pool(name="cq", bufs=1) as cq_pool,
                tc.tile_pool(name="wa", bufs=1) as wa_pool,
            ):
                cq = cq_pool.tile([128, DT, T], BF16)
                wa = wa_pool.tile([128, KT, DT * 128], BF16)
                with (
                    tc.tile_pool(name="wc", bufs=2) as wc_pool,
                    tc.tile_pool(name="ex", bufs=2) as ex_pool,
                ):
                    for a in range(DT):
                        wc = wc_pool.tile([128, KT, 3 * 128], BF16, tag="wc")
                        nc.sync.dma_start(
                            wc[:],
                            wct[a].rearrange("(kc p) km -> p kc km", p=128),
                        )
                        if a == 0:
                            nc.sync.dma_start(
                                wog[:].rearrange("p kc a m -> p kc (a m)"),
                                wqt[:, 2 * D:3 * D]
                                .rearrange("(kc p) m -> p kc m", p=128),
                            )
                        if a == 2:
                            nc.sync.dma_start(
                                wa[:],
                                wat[:].rearrange("(kc p) m -> p kc m", p=128),
                            )
                        if a == 4:
                            nc.sync.dma_start(
                                wo2[:].rearrange("p kc a m -> p kc (a m)"),
                                wot[:].rearrange("(kc p) m -> p kc m", p=128),
                            )
                        for blk in range(T // BBLK):
                            t0 = blk * BBLK
                            ps = psu.tile([128, BBLK], F32, tag="ps")
                            first = True
                            for k3 in range(3):
                                for kc in range(KT):
                                    nc.tensor.matmul(
                                        ps[:],
                                        wc[:, kc, k3 * 128:(k3 + 1) * 128],
                                        qt[:, kc, t0 + k3:t0 + k3 + BBLK],
                                        start=first,
                                        stop=(k3 == 2 and kc == KT - 1),
                                    )
                                    first = False
                            sig = wc_pool.tile([128, BBLK], F32, tag="sig")
                            nc.scalar.activation(
                                sig[:], ps[:], AF.Sigmoid, bias=cb_sb[:, a:a + 1]
                            )
                            nc.vector.scalar_tensor_tensor(
                                cq[:, a, t0:t0 + BBLK], ps[:], cb_sb[:, a:a + 1],
                                sig[:], OP.add, OP.mult,
                            )

                    # ------- phase B2: E/N partial sums from exp(logits) ----
                    # (same PSUM pool as B1 so the bank rotation pipelines
                    # straight across the phase boundary)
                    for blk in range(T // BBLK):
                        t0 = blk * BBLK
                        for a in range(DT):
                            ps = psu.tile([128, BBLK], F32, tag="ps")
                            for kc in range(KT):
                                nc.tensor.matmul(
                                    ps[:], wa[:, kc, a * 128:(a + 1) * 128],
                                    cq[:, kc, t0:t0 + BBLK],
                                    start=(kc == 0), stop=(kc == KT - 1),
                                )
                            expl = ex_pool.tile([128, BBLK], F32, tag="expl")
                            idx = a * 4 + blk
                            nc.scalar.activation(
                                expl[:], ps[:], AF.Exp, scale=SCALE,
                                accum_out=e_cols[:, idx:idx + 1],
                            )
                            prod = ex_pool.tile([128, BBLK], BF16, tag="prod")
                            nc.vector.scalar_tensor_tensor(
                                prod[:], expl[:], 0.0,
                                qt[:, a, t0 + 1:t0 + 1 + BBLK],
                                OP.add, OP.mult,
                                accum_out=n_cols[:, idx:idx + 1],
                            )

        # -------- offv = hf0 * (W_O @ sx + T*b_O), before the collective ----
        # (uses only local sx, which is exact on first-half cores and masked
        # to zero on second-half ones)
        bo_t = cols.tile([128, DT], F32)
        nc.vector.tensor_scalar_mul(bo_t[:], bo_sb[:], float(T))
        if True:
            for a in range(DT):
                ps = psu.tile([128, 1], F32, tag="ps")
                for kc in range(KT):
                    nc.tensor.matmul(
                        ps[:], woo[:, kc, a, :], sxb[:, kc:kc + 1],
                        start=(kc == 0), stop=(kc == KT - 1),
                    )
                nc.vector.tensor_scalar_add(
                    stage[:, 2 * DT + a:2 * DT + a + 1], ps[:], bo_t[:, a:a + 1]
                )
        nc.vector.tensor_scalar_mul(
            stage[:, 2 * DT:3 * DT], stage[:, 2 * DT:3 * DT], hf0_sb[:, 0:1]
        )

        # ---------------- allreduce E, N, offv over the seq pair ----------------
        nc.vector.tensor_reduce(
            stage[:, 0:DT], e_cols[:].rearrange("p (a b) -> p a b", b=4),
            axis=mybir.AxisListType.X, op=OP.add,
        )
        nc.vector.tensor_reduce(
            stage[:, DT:2 * DT], n_cols[:].rearrange("p (a b) -> p a b", b=4),
            axis=mybir.AxisListType.X, op=OP.add,
        )
        if phases == 99:
            # timing-model variant: skip the collective (TimelineSim
            # cannot model collectives); copy stage -> red locally
            nc.vector.tensor_copy(red[:], stage[:])
        else:
            cc_in = dram.tile([128, 3 * DT], F32)
            cc_out = dram.tile([128, 3 * DT], F32)
            nc.sync.dma_start(cc_in[:], stage[:])
            nc.gpsimd.collective_compute(
                "AllReduce", OP.add,
                replica_groups=[[0, 1], [2, 3], [4, 5], [6, 7]],
                ins=[cc_in.opt()], outs=[cc_out.opt()],
            )
            nc.sync.dma_start(red[:], cc_out[:])

        # glob = N / E ; offset = glob * offv * hf1 ; boglob = b_o * glob
        recip = cols.tile([128, DT], F32)
        nc.vector.reciprocal(recip[:], red[:, 0:DT])
        nc.vector.tensor_mul(glob[:], red[:, DT:2 * DT], recip[:])
        nc.vector.tensor_mul(offset[:], red[:, 2 * DT:3 * DT], glob[:])
        nc.vector.tensor_scalar_mul(offset[:], offset[:], hf1_sb[:, 0:1])
        nc.vector.tensor_mul(boglob[:], bo_sb[:], glob[:])

        # ---------------- phase C: O,G -> P -> cumsum -> L -> R ----------------
        # software-pipelined: PE order per iteration is O_i, R_{i-1}, G_i so
        # the W_out matmul of block i runs while block i+1's silu/cumsum
        # chain completes on DVE/ACT
        with tc.tile_pool(name="blkb", bufs=2) as blk_pool:
            nblk = T // CBLK
            c_prev = None
            hist = []          # (lt, rt) of the previous block
            for blk in range(nblk):
                t0 = blk * CBLK
                pt = blk_pool.tile([128, DT, CBLK], F32, tag="pt")
                ct = blk_pool.tile([128, DT, CBLK], F32, tag="ct")
                carry = blk_pool.tile([128, DT], F32, tag="carry")
                gt = blk_pool.tile([128, DT, CBLK], BF16, tag="gt")
                lt = blk_pool.tile([128, DT, CBLK], BF16, tag="lt")
                # O-projection + P + cumsum for this block
                for a in range(DT):
                    ps = psu.tile([128, CBLK], F32, tag="ps")
                    for kc in range(KT):
                        nc.tensor.matmul(
                            ps[:], woo[:, kc, a, :], xa[:, kc, t0 + 1:t0 + 1 + CBLK],
                            start=(kc == 0), stop=(kc == KT - 1),
                        )
                    # P = (O + b_o) * glob = O*glob + (b_o*glob)
                    nc.vector.tensor_scalar(
                        pt[:, a, :], ps[:], glob[:, a:a + 1], boglob[:, a:a + 1],
                        OP.mult, OP.add,
                    )
                    init = (offset[:, a:a + 1] if c_prev is None
                            else c_prev[:, a:a + 1])
                    nc.vector.tensor_tensor_scan(
                        ct[:, a, :], pt[:, a, :], pt[:, a, :], init,
                        OP.add, OP.bypass,
                    )
                # carry the last cumsum column via ACT so the next
                # block's scan does not read a scan output directly
                nc.scalar.copy(carry[:], ct[:, :, CBLK - 1:CBLK])
                # output matmul of the PREVIOUS block
                if hist:
                    plt, prt, pt0 = hist.pop()
                    for a in range(DT):
                        ps = psu.tile([128, CBLK], F32, tag="ps")
                        for kc in range(KT):
                            nc.tensor.matmul(
                                ps[:], wo2[:, kc, a, :], plt[:, kc, :],
                                start=(kc == 0), stop=(kc == KT - 1),
                            )
                        nc.scalar.activation(
                            prt[:, a, :], ps[:], AF.Identity,
                            bias=bout_sb[:, a:a + 1],
                        )
                        nc.sync.dma_start(
                            r_out[a * 128:(a + 1) * 128, pt0:pt0 + CBLK],
                            prt[:, a, :],
                        )
                # G-projection + silu + L for this block
                for a in range(DT):
                    ps = psu.tile([128, CBLK], F32, tag="ps")
                    for kc in range(KT):
                        nc.tensor.matmul(
                            ps[:], wog[:, kc, a, :], xa[:, kc, t0 + 1:t0 + 1 + CBLK],
                            start=(kc == 0), stop=(kc == KT - 1),
                        )
                    sig = blk_pool.tile([128, CBLK], F32, tag="sig")
                    nc.scalar.activation(
                        sig[:], ps[:], AF.Sigmoid, bias=bg_sb[:, a:a + 1]
                    )
                    nc.vector.scalar_tensor_tensor(
                        gt[:, a, :], ps[:], bg_sb[:, a:a + 1], sig[:],
                        OP.add, OP.mult,
                    )
                    nc.vector.tensor_mul(lt[:, a, :], gt[:, a, :], ct[:, a, :])
                rt = blk_pool.tile([128, DT, CBLK], BF16, tag="rt")
                hist.append((lt, rt, t0))
                c_prev = carry
            # drain: output matmul of the final block; the bias add
            # alternates ACT/DVE so neither engine queues up a tail
            plt, prt, pt0 = hist.pop()
            for a in range(DT):
                ps = psu.tile([128, CBLK], F32, tag="ps")
                for kc in range(KT):
                    nc.tensor.matmul(
                        ps[:], wo2[:, kc, a, :], plt[:, kc, :],
                        start=(kc == 0), stop=(kc == KT - 1),
                    )
                if a % 2 == 0:
                    nc.scalar.activation(
                        prt[:, a, :], ps[:], AF.Identity,
                        bias=bout_sb[:, a:a + 1],
                    )
                else:
                    nc.vector.tensor_scalar_add(
                        prt[:, a, :], ps[:], bout_sb[:, a:a + 1]
                    )
                nc.sync.dma_start(
                    r_out[a * 128:(a + 1) * 128, pt0:pt0 + CBLK],
                    prt[:, a, :],
                )


_CACHE = {}


def _build(phases=5):
    if phases in _CACHE:
        return _CACHE[phases]
    nc = bacc.Bacc(None, target_bir_lowering=False, num_devices=N_CORES)
    prm = {
        "wall": nc.declare_dram_parameter("wall", [NWALL], BF16, isOutput=False),
        "r": nc.declare_dram_parameter("r", [DM, T], BF16, isOutput=True),
    }
    with tile.TileContext(nc, num_cores=N_CORES) as tc:
        _emit(tc, nc, prm, phases)
    nc.compile()
    _CACHE[phases] = nc
    return nc


def make_in_maps(x, W_qog, b_qog, conv_w, conv_b, w_a, W_out, b_out):
    f = np.float32
    bf = ml_dtypes.bfloat16
    x = np.asarray(x, f)
    wqt = np.asarray(W_qog, f).T.astype(bf)                          # [dm, 3d]
    # conv_w [o, i, k] -> wct [a, i, k*128+m] with o = a*128 + m, so one DMA
    # per output tile loads the whole 3-tap stationary slab
    wct = (
        np.asarray(conv_w, f)
        .reshape(DT, 128, D, 3)
        .transpose(0, 2, 3, 1)
        .reshape(DT, D, 3 * 128)
        .astype(bf)
    )
    wat = np.asarray(w_a, f).T.astype(bf)
    wot = np.asarray(W_out, f).T.astype(bf)

    def col(v):  # [d] -> [128, DT] with d = a*128 + p
        return np.asarray(v, f).reshape(DT, 128).T

    b_qog = np.asarray(b_qog, f)
    bcol = np.concatenate(
        [col(b_qog[:D]), col(b_qog[D:2 * D]), col(b_qog[2 * D:]),
         col(conv_b), col(b_out)], axis=1)
    wtail = np.concatenate(
        [wqt.reshape(-1), wct.reshape(-1), wat.reshape(-1), wot.reshape(-1)])

    in_maps = []
    for c in range(N_CORES):
        b, h = c // 2, c % 2
        t0 = h * T
        xs = np.zeros((TH, DM), f)
        xs[1:T + 1] = x[b, t0:t0 + T]
        if t0 > 0:
            xs[0] = x[b, t0 - 1]
        if t0 + T < S:
            xs[T + 1] = x[b, t0 + T]
        xs = xs.T.astype(bf)                         # [DM, TH] feature-major
        bh = np.concatenate(
            [bcol,
             np.full((128, 1), 1.0 - h, f),
             np.full((128, 1), float(h), f)], axis=1).astype(bf)
        wall = np.concatenate(
            [xs.reshape(-1), bh.reshape(-1), wtail])
        assert wall.size == NWALL
        in_maps.append({"wall": np.ascontiguousarray(wall)})
    return in_maps


def kernel(x, W_qog, b_qog, conv_w, conv_b, w_a, W_out, b_out):
    nc = _build(5)
    in_maps = make_in_maps(x, W_qog, b_qog, conv_w, conv_b, w_a, W_out, b_out)
    res = None
    for attempt in range(3):
        try:
            res = run_bass_kernel_spmd(nc, in_maps, list(range(N_CORES)))
            break
        except Exception:
            # the execution path through the device bridge is occasionally
            # flaky (worker hangup); reset the backend and retry
            if attempt == 2:
                raise
            import jax

            try:
                jax.clear_backends()
            except Exception:
                pass
            import time

            time.sleep(5)
    out = np.empty((B, S, DM), np.float32)
    for c in range(N_CORES):
        b, h = c // 2, c % 2
        out[b, h * T:(h + 1) * T, :] = np.asarray(res.results[c]["r"], np.float32).T
    return out


# revision 28
# speedup vs baseline: 11.5095x; 1.0219x over previous
"""F2NetHead Trainium2 kernel (8 NeuronCores, Bass/Tile).

Reference computation (per batch b):
    qog = x @ W_qog.T + b_qog ; Q,O,G = split(qog)
    cq  = silu(conv1d(Q, conv_w, pad=1) + conv_b)          # mixes channels
    l   = (cq @ w_a.T) / sqrt(d)
    attn= softmax(l, axis=seq)
    glob= sum_seq(Q * attn)                                 # [1, d]
    P   = O * glob
    L   = silu(G) * cumsum(P, axis=seq)
    R   = L @ W_out.T + b_out

Sharding: 8 cores = 4 batches x 2 sequence halves. Each core computes
2048 tokens of one batch. The host supplies the x-rows with a 1-token
halo on each side (zero rows at the sequence edges) so the conv needs no
neighbor exchange. The only cross-core communication is a pairwise
AllReduce of 3 small [d] vectors per batch:
    E    = sum_seq exp(l)            (softmax denominator)
    N    = sum_seq Q * exp(l)        (softmax numerator of glob)
    offv = hf0 * (W_O @ sx + T*b_O)  (first half's P-column-sums / glob)
The cumsum offset of the second half is glob * offv, computed BEFORE the
collective from the local x column-sums (masked to the first half) so the
tensor engine's program order never stalls on the allreduce: after the
B2 matmuls it proceeds straight into phase C's O/G matmuls, which only
need weights prefetched long before.

On-chip layout is feature-major ([d partitions, tokens free]) so every
sequence-axis op (softmax sums, global sum, cumsum) is a free-dim op.
All matmul operands are bf16 (same PE rate as fp32r but half the DMA
and SBUF footprint, which is what lets every weight prefetch early and
x stay resident); all accumulations (PSUM, softmax sums, cumsum) are
fp32. Phase C is software-pipelined one block deep (PE order per block:
O_i, R_{i-1}, G_i) so the output matmul never waits on the silu/cumsum
chain of its own block.
"""

import numpy as np
import ml_dtypes

import concourse.bacc as bacc
import concourse.mybir as mybir
import concourse.tile as tile
from concourse.bass_utils import run_bass_kernel_spmd

F32 = mybir.dt.float32
BF16 = mybir.dt.bfloat16
AF = mybir.ActivationFunctionType
OP = mybir.AluOpType

B, S, D, DM = 4, 4096, 1024, 1024
N_CORES = 8
T = S // 2            # tokens per core
TH = T + 2            # with halo
DT = D // 128         # d tiles (8)
KT = DM // 128        # contraction tiles (8)
ABLK = 410            # phase A token block (5 blocks over TH=2050)
BBLK = 512            # phase B token block (4 blocks over T)
CBLK = 512            # phase C token block (4 blocks over T)
SCALE = 1.0 / float(np.sqrt(D))

# every input rides in one flat bf16 tensor (fewer per-launch dispatch args);
# region offsets in elements
OFF_X = 0                          # x        [DM, TH]
OFF_BH = OFF_X + DM * TH           # bcol|hf  [128, 5*DT + 2]
OFF_WQ = OFF_BH + 128 * (5 * DT + 2)   # W_qog^T  [DM, 3*D]
OFF_WC = OFF_WQ + DM * 3 * D       # conv     [DT, D, 3*128]
OFF_WA = OFF_WC + DT * D * 3 * 128  # w_a^T   [D, D]
OFF_WO = OFF_WA + D * D            # W_out^T  [D, D]
NWALL = OFF_WO + D * D


def _emit(tc, nc, prm, phases=5):
    reps = 1
    if phases >= 100:
        reps, phases = phases // 100, 5
    for _ in range(reps):
        _emit_once(tc, nc, prm, phases)


def _emit_once(tc, nc, prm, phases):
    wall, r_out = prm["wall"], prm["r"]
    xr = wall[OFF_X:OFF_BH].rearrange("(kc p t) -> p kc t", p=128, t=TH)
    bh = wall[OFF_BH:OFF_WQ].rearrange("(p c) -> p c", c=5 * DT + 2)
    wqt = wall[OFF_WQ:OFF_WC].rearrange("(i m) -> i m", m=3 * D)
    wct = wall[OFF_WC:OFF_WA].rearrange("(a i km) -> a i km", i=D, km=3 * 128)
    wat = wall[OFF_WA:OFF_WO].rearrange("(i m) -> i m", m=D)
    wot = wall[OFF_WO:NWALL].rearrange("(i m) -> i m", m=D)

    with (
        tc.tile_pool(name="cols", bufs=1) as cols,
        tc.tile_pool(name="xres", bufs=1) as x_pool,
        tc.tile_pool(name="woo", bufs=1) as woo_pool,
        tc.tile_pool(name="wog", bufs=1) as wog_pool,
        tc.tile_pool(name="wo2", bufs=1) as wo2_pool,
        tc.tile_pool(name="psu", bufs=8, space="PSUM") as psu,
        tc.tile_pool(name="dram", bufs=1, space="DRAM") as dram,
    ):
        # x stays resident for the whole kernel: phase A consumes it by
        # blocks, phase C's O/G matmuls reread it with no second DMA
        xa = x_pool.tile([128, KT, TH], BF16)
        woo = woo_pool.tile([128, KT, DT, 128], BF16)
        wog = wog_pool.tile([128, KT, DT, 128], BF16)
        wo2 = wo2_pool.tile([128, KT, DT, 128], BF16)

        # per-partition bias / flag columns ([128, DT] with d = a*128 + p),
        # packed [bq|bo|bg|cb|bout|hf0|hf1]: one DMA + one widening copy
        ball_bf = cols.tile([128, 5 * DT + 2], BF16)
        ball = cols.tile([128, 5 * DT + 2], F32)
        bq_sb = ball[:, 0:DT]
        bo_sb = ball[:, DT:2 * DT]
        bg_sb = ball[:, 2 * DT:3 * DT]
        cb_sb = ball[:, 3 * DT:4 * DT]
        bout_sb = ball[:, 4 * DT:5 * DT]
        hf0_sb = ball[:, 5 * DT:5 * DT + 1]
        hf1_sb = ball[:, 5 * DT + 1:5 * DT + 2]

        # accumulators that survive across phases
        sx_cols = cols.tile([128, KT], F32)         # x column sums (main toks)
        sxb = cols.tile([128, KT], BF16)            # ... as matvec operand
        e_cols = cols.tile([128, DT * 4], F32)      # per-(a,B-block) exp sums
        n_cols = cols.tile([128, DT * 4], F32)      # per-(a,B-block) Q*exp sums
        stage = cols.tile([128, 3 * DT], F32)       # allreduce staging
        red = cols.tile([128, 3 * DT], F32)         # allreduce result
        glob = cols.tile([128, DT], F32)
        offset = cols.tile([128, DT], F32)
        boglob = cols.tile([128, DT], F32)

        # ---------------- phase A: Q^T over TH halo'd tokens ----------------
        # DMA queue order is emission order, so criticals go first: x block 0
        # and wq feed the first matmuls; everything phase C needs trickles in
        # behind the phase A stream.
        with tc.tile_pool(name="qt", bufs=1) as qt_pool:
            qt = qt_pool.tile([128, DT, TH], BF16)
            with tc.tile_pool(name="wq", bufs=1) as wq_pool:
                wqr = wqt[:, 0:D].rearrange("(kc p) m -> p kc m", p=128)
                wq = wq_pool.tile([128, KT, DT * 128], BF16)
                # interleave the first x block with wq so the a=0 matmul
                # chain can start as soon as its first operands land
                for kc in (0, 1):
                    nc.sync.dma_start(
                        xa[:, kc:kc + 1, 0:ABLK], xr[:, kc:kc + 1, 0:ABLK]
                    )
                    nc.sync.dma_start(wq[:, kc:kc + 1, :], wqr[:, kc:kc + 1, :])
                for kc in range(2, KT, 2):
                    nc.sync.dma_start(
                        xa[:, kc:kc + 2, 0:ABLK], xr[:, kc:kc + 2, 0:ABLK]
                    )
                    nc.sync.dma_start(wq[:, kc:kc + 2, :], wqr[:, kc:kc + 2, :])
                    if kc == 2:
                        # biases aren't needed until the first qt write
                        nc.sync.dma_start(ball_bf[:], bh[:])
                        nc.vector.tensor_copy(ball[:], ball_bf[:])
                for blk in range(5):
                    t0 = blk * ABLK
                    if blk > 0:
                        nc.sync.dma_start(
                            xa[:, :, t0:t0 + ABLK], xr[:, :, t0:t0 + ABLK]
                        )
                    for a in range(DT):
                        ps = psu.tile([128, ABLK], F32, tag="ps")
                        for kc in range(KT):
                            nc.tensor.matmul(
                                ps[:], wq[:, kc, a * 128:(a + 1) * 128],
                                xa[:, kc, t0:t0 + ABLK],
                                start=(kc == 0), stop=(kc == KT - 1),
                            )
                        nc.vector.tensor_scalar_add(
                            qt[:, a, t0:t0 + ABLK], ps[:], bq_sb[:, a:a + 1]
                        )
                    if blk == 0:
                        # phase C's O-projection weights ride behind block 0
                        nc.sync.dma_start(
                            woo[:].rearrange("p kc a m -> p kc (a m)"),
                            wqt[:, D:2 * D].rearrange("(kc p) m -> p kc m", p=128),
                        )
                # x column sums over main tokens, for the cumsum offset
                for kc in range(KT):
                    nc.vector.tensor_reduce(
                        sx_cols[:, kc:kc + 1], xa[:, kc, 1:T + 1],
                        axis=mybir.AxisListType.X, op=OP.add,
                    )
                nc.vector.tensor_copy(sxb[:], sx_cols[:])

            # ------------- phase B1: cq^T = silu(conv(Q)) -------------
            with (
                tc.tile_pool(name="cq", bufs=1) as cq_pool,
                tc.tile_pool(name="wa", bufs=1) as wa_pool,
            ):
                cq = cq_pool.tile([128, DT, T], BF16)
                wa = wa_pool.tile([128, KT, DT * 128], BF16)
                with (
                    tc.tile_pool(name="wc", bufs=2) as wc_pool,
                    tc.tile_pool(name="ex", bufs=2) as ex_pool,
                ):
                    for a in range(DT):
                        wc = wc_pool.tile([128, KT, 3 * 128], BF16, tag="wc")
                        nc.sync.dma_start(
                            wc[:],
                            wct[a].rearrange("(kc p) km -> p kc km", p=128),
                        )
                        if a == 0:
                            nc.sync.dma_start(
                                wog[:].rearrange("p kc a m -> p kc (a m)"),
                                wqt[:, 2 * D:3 * D]
                                .rearrange("(kc p) m -> p kc m", p=128),
                            )
                        if a == 2:
                            nc.sync.dma_start(
                                wa[:],
                                wat[:].rearrange("(kc p) m -> p kc m", p=128),
                            )
                        if a == 4:
                            nc.sync.dma_start(
                                wo2[:].rearrange("p kc a m -> p kc (a m)"),
                                wot[:].rearrange("(kc p) m -> p kc m", p=128),
                            )
                        for blk in range(T // BBLK):
                            t0 = blk * BBLK
                            ps = psu.tile([128, BBLK], F32, tag="ps")
                            first = True
                            for k3 in range(3):
                                for kc in range(KT):
                                    nc.tensor.matmul(
                                        ps[:],
                                        wc[:, kc, k3 * 128:(k3 + 1) * 128],
                                        qt[:, kc, t0 + k3:t0 + k3 + BBLK],
                                        start=first,
                                        stop=(k3 == 2 and kc == KT - 1),
                                    )
                                    first = False
                            sig = wc_pool.tile([128, BBLK], F32, tag="sig")
                            nc.scalar.activation(
                                sig[:], ps[:], AF.Sigmoid, bias=cb_sb[:, a:a + 1]
                            )
                            nc.vector.scalar_tensor_tensor(
                                cq[:, a, t0:t0 + BBLK], ps[:], cb_sb[:, a:a + 1],
                                sig[:], OP.add, OP.mult,
                            )

                    # ------- phase B2: E/N partial sums from exp(logits) ----
                    # (same PSUM pool as B1 so the bank rotation pipelines
                    # straight across the phase boundary)
                    bo_t = cols.tile([128, DT], F32)
                    nc.vector.tensor_scalar_mul(bo_t[:], bo_sb[:], float(T))
                    for blk in range(T // BBLK):
                        t0 = blk * BBLK
                        for a in range(DT):
                            ps = psu.tile([128, BBLK], F32, tag="ps")
                            for kc in range(KT):
                                nc.tensor.matmul(
                                    ps[:], wa[:, kc, a * 128:(a + 1) * 128],
                                    cq[:, kc, t0:t0 + BBLK],
                                    start=(kc == 0), stop=(kc == KT - 1),
                                )
                            expl = ex_pool.tile([128, BBLK], F32, tag="expl")
                            idx = a * 4 + blk
                            nc.scalar.activation(
                                expl[:], ps[:], AF.Exp, scale=SCALE,
                                accum_out=e_cols[:, idx:idx + 1],
                            )
                            prod = ex_pool.tile([128, BBLK], BF16, tag="prod")
                            nc.vector.scalar_tensor_tensor(
                                prod[:], expl[:], 0.0,
                                qt[:, a, t0 + 1:t0 + 1 + BBLK],
                                OP.add, OP.mult,
                                accum_out=n_cols[:, idx:idx + 1],
                            )
                            if blk == T // BBLK - 1:
                                # offv = W_O @ sx + T*b_O, one matvec group
                                # per B2 group: interleaved, so each reuses a
                                # PSUM bank whose reader finished long ago and
                                # the PE stream never stalls to catch up
                                psv = psu.tile([128, 1], F32, tag="ps")
                                for kc in range(KT):
                                    nc.tensor.matmul(
                                        psv[:], woo[:, kc, a, :],
                                        sxb[:, kc:kc + 1],
                                        start=(kc == 0), stop=(kc == KT - 1),
                                    )
                                nc.vector.tensor_scalar_add(
                                    stage[:, 2 * DT + a:2 * DT + a + 1],
                                    psv[:], bo_t[:, a:a + 1],
                                )

        # offv is masked to the first half before the allreduce
        nc.vector.tensor_scalar_mul(
            stage[:, 2 * DT:3 * DT], stage[:, 2 * DT:3 * DT], hf0_sb[:, 0:1]
        )

        # ---------------- allreduce E, N, offv over the seq pair ----------------
        nc.vector.tensor_reduce(
            stage[:, 0:DT], e_cols[:].rearrange("p (a b) -> p a b", b=4),
            axis=mybir.AxisListType.X, op=OP.add,
        )
        nc.vector.tensor_reduce(
            stage[:, DT:2 * DT], n_cols[:].rearrange("p (a b) -> p a b", b=4),
            axis=mybir.AxisListType.X, op=OP.add,
        )
        if phases == 99:
            # timing-model variant: skip the collective (TimelineSim
            # cannot model collectives); copy stage -> red locally
            nc.vector.tensor_copy(red[:], stage[:])
        else:
            cc_in = dram.tile([128, 3 * DT], F32)
            cc_out = dram.tile([128, 3 * DT], F32)
            nc.sync.dma_start(cc_in[:], stage[:])
            nc.gpsimd.collective_compute(
                "AllReduce", OP.add,
                replica_groups=[[0, 1], [2, 3], [4, 5], [6, 7]],
                ins=[cc_in.opt()], outs=[cc_out.opt()],
            )
            nc.sync.dma_start(red[:], cc_out[:])

        # glob = N / E ; offset = glob * offv * hf1 ; boglob = b_o * glob
        recip = cols.tile([128, DT], F32)
        nc.vector.reciprocal(recip[:], red[:, 0:DT])
        nc.vector.tensor_mul(glob[:], red[:, DT:2 * DT], recip[:])
        nc.vector.tensor_mul(offset[:], red[:, 2 * DT:3 * DT], glob[:])
        nc.vector.tensor_scalar_mul(offset[:], offset[:], hf1_sb[:, 0:1])
        nc.vector.tensor_mul(boglob[:], bo_sb[:], glob[:])

        # ---------------- phase C: O,G -> P -> cumsum -> L -> R ----------------
        # software-pipelined: PE order per iteration is O_i, R_{i-1}, G_i so
        # the W_out matmul of block i runs while block i+1's silu/cumsum
        # chain completes on DVE/ACT
        with tc.tile_pool(name="blkb", bufs=2) as blk_pool:
            nblk = T // CBLK
            c_prev = None
            hist = []          # (lt, rt) of the previous block
            for blk in range(nblk):
                t0 = blk * CBLK
                pt = blk_pool.tile([128, DT, CBLK], F32, tag="pt")
                ct = blk_pool.tile([128, DT, CBLK], F32, tag="ct")
                carry = blk_pool.tile([128, DT], F32, tag="carry")
                gt = blk_pool.tile([128, DT, CBLK], BF16, tag="gt")
                lt = blk_pool.tile([128, DT, CBLK], BF16, tag="lt")
                # O-projection + P + cumsum for this block
                for a in range(DT):
                    ps = psu.tile([128, CBLK], F32, tag="ps")
                    for kc in range(KT):
                        nc.tensor.matmul(
                            ps[:], woo[:, kc, a, :], xa[:, kc, t0 + 1:t0 + 1 + CBLK],
                            start=(kc == 0), stop=(kc == KT - 1),
                        )
                    # P = (O + b_o) * glob = O*glob + (b_o*glob)
                    nc.vector.tensor_scalar(
                        pt[:, a, :], ps[:], glob[:, a:a + 1], boglob[:, a:a + 1],
                        OP.mult, OP.add,
                    )
                    init = (offset[:, a:a + 1] if c_prev is None
                            else c_prev[:, a:a + 1])
                    nc.vector.tensor_tensor_scan(
                        ct[:, a, :], pt[:, a, :], pt[:, a, :], init,
                        OP.add, OP.bypass,
                    )
                # carry the last cumsum column via ACT so the next
                # block's scan does not read a scan output directly
                nc.scalar.copy(carry[:], ct[:, :, CBLK - 1:CBLK])
                # output matmul of the PREVIOUS block
                if hist:
                    plt, prt, pt0 = hist.pop()
                    for a in range(DT):
                        ps = psu.tile([128, CBLK], F32, tag="ps")
                        for kc in range(KT):
                            nc.tensor.matmul(
                                ps[:], wo2[:, kc, a, :], plt[:, kc, :],
                                start=(kc == 0), stop=(kc == KT - 1),
                            )
                        nc.scalar.activation(
                            prt[:, a, :], ps[:], AF.Identity,
                            bias=bout_sb[:, a:a + 1],
                        )
                        nc.sync.dma_start(
                            r_out[a * 128:(a + 1) * 128, pt0:pt0 + CBLK],
                            prt[:, a, :],
                        )
                # G-projection + silu + L for this block
                for a in range(DT):
                    ps = psu.tile([128, CBLK], F32, tag="ps")
                    for kc in range(KT):
                        nc.tensor.matmul(
                            ps[:], wog[:, kc, a, :], xa[:, kc, t0 + 1:t0 + 1 + CBLK],
                            start=(kc == 0), stop=(kc == KT - 1),
                        )
                    sig = blk_pool.tile([128, CBLK], F32, tag="sig")
                    nc.scalar.activation(
                        sig[:], ps[:], AF.Sigmoid, bias=bg_sb[:, a:a + 1]
                    )
                    nc.vector.scalar_tensor_tensor(
                        gt[:, a, :], ps[:], bg_sb[:, a:a + 1], sig[:],
                        OP.add, OP.mult,
                    )
                    nc.vector.tensor_mul(lt[:, a, :], gt[:, a, :], ct[:, a, :])
                rt = blk_pool.tile([128, DT, CBLK], BF16, tag="rt")
                hist.append((lt, rt, t0))
                c_prev = carry
            # drain: output matmul of the final block, in half-blocks
            # with the bias add alternating ACT/DVE, so the post-matmul
            # tail is one 256-token activation + DMA instead of a full
            # block's worth
            plt, prt, pt0 = hist.pop()
            HB = CBLK // 2
            for a in range(DT):
                for h in range(2):
                    ps = psu.tile([128, HB], F32, tag="ps")
                    for kc in range(KT):
                        nc.tensor.matmul(
                            ps[:], wo2[:, kc, a, :],
                            plt[:, kc, h * HB:(h + 1) * HB],
                            start=(kc == 0), stop=(kc == KT - 1),
                        )
                    if (a + h) % 2 == 0:
                        nc.scalar.activation(
                            prt[:, a, h * HB:(h + 1) * HB], ps[:], AF.Identity,
                            bias=bout_sb[:, a:a + 1],
                        )
                    else:
                        nc.vector.tensor_scalar_add(
                            prt[:, a, h * HB:(h + 1) * HB], ps[:],
                            bout_sb[:, a:a + 1]
                        )
                    nc.sync.dma_start(
                        r_out[a * 128:(a + 1) * 128,
                              pt0 + h * HB:pt0 + (h + 1) * HB],
                        prt[:, a, h * HB:(h + 1) * HB],
                    )


_CACHE = {}


def _build(phases=5):
    if phases in _CACHE:
        return _CACHE[phases]
    nc = bacc.Bacc(None, target_bir_lowering=False, num_devices=N_CORES)
    prm = {
        "wall": nc.declare_dram_parameter("wall", [NWALL], BF16, isOutput=False),
        "r": nc.declare_dram_parameter("r", [DM, T], BF16, isOutput=True),
    }
    with tile.TileContext(nc, num_cores=N_CORES) as tc:
        _emit(tc, nc, prm, phases)
    nc.compile()
    _CACHE[phases] = nc
    return nc


def make_in_maps(x, W_qog, b_qog, conv_w, conv_b, w_a, W_out, b_out):
    f = np.float32
    bf = ml_dtypes.bfloat16
    x = np.asarray(x, f)
    wqt = np.asarray(W_qog, f).T.astype(bf)                          # [dm, 3d]
    # conv_w [o, i, k] -> wct [a, i, k*128+m] with o = a*128 + m, so one DMA
    # per output tile loads the whole 3-tap stationary slab
    wct = (
        np.asarray(conv_w, f)
        .reshape(DT, 128, D, 3)
        .transpose(0, 2, 3, 1)
        .reshape(DT, D, 3 * 128)
        .astype(bf)
    )
    wat = np.asarray(w_a, f).T.astype(bf)
    wot = np.asarray(W_out, f).T.astype(bf)

    def col(v):  # [d] -> [128, DT] with d = a*128 + p
        return np.asarray(v, f).reshape(DT, 128).T

    b_qog = np.asarray(b_qog, f)
    bcol = np.concatenate(
        [col(b_qog[:D]), col(b_qog[D:2 * D]), col(b_qog[2 * D:]),
         col(conv_b), col(b_out)], axis=1)
    wtail = np.concatenate(
        [wqt.reshape(-1), wct.reshape(-1), wat.reshape(-1), wot.reshape(-1)])

    in_maps = []
    for c in range(N_CORES):
        b, h = c // 2, c % 2
        t0 = h * T
        xs = np.zeros((TH, DM), f)
        xs[1:T + 1] = x[b, t0:t0 + T]
        if t0 > 0:
            xs[0] = x[b, t0 - 1]
        if t0 + T < S:
            xs[T + 1] = x[b, t0 + T]
        xs = xs.T.astype(bf)                         # [DM, TH] feature-major
        bh = np.concatenate(
            [bcol,
             np.full((128, 1), 1.0 - h, f),
             np.full((128, 1), float(h), f)], axis=1).astype(bf)
        wall = np.concatenate(
            [xs.reshape(-1), bh.reshape(-1), wtail])
        assert wall.size == NWALL
        in_maps.append({"wall": np.ascontiguousarray(wall)})
    return in_maps


def kernel(x, W_qog, b_qog, conv_w, conv_b, w_a, W_out, b_out):
    nc = _build(5)
    in_maps = make_in_maps(x, W_qog, b_qog, conv_w, conv_b, w_a, W_out, b_out)
    res = None
    for attempt in range(3):
        try:
            res = run_bass_kernel_spmd(nc, in_maps, list(range(N_CORES)))
            break
        except Exception:
            # the execution path through the device bridge is occasionally
            # flaky (worker hangup); reset the backend and retry
            if attempt == 2:
                raise
            import jax

            try:
                jax.clear_backends()
            except Exception:
                pass
            import time

            time.sleep(5)
    out = np.empty((B, S, DM), np.float32)
    for c in range(N_CORES):
        b, h = c // 2, c % 2
        out[b, h * T:(h + 1) * T, :] = np.asarray(res.results[c]["r"], np.float32).T
    return out
